# revision 16
# baseline (speedup 1.0000x reference)
"""GCNBlock (2-graph GCN, depth 4) on 8 Trainium2 NeuronCores.

Strategy (graph/data parallel, per sharding hint):
  - Nodes row-sharded 8 ways (6250/core, padded to 6272 = 49 blocks of 128).
  - Per layer: every core aggregates messages for its own 6250 dst nodes by
    DMA-gathering per-edge source rows (fp16) from a replicated node-feature
    table in HBM, reduces them into per-block sums with PE matmuls against
    per-chunk selection matrices (built on DVE: (iota == dst_local) * norm),
    applies the two [F,F] GEMMs + bias + ReLU, then AllGathers the new
    feature shard so every core has the full table for the next layer.
  - dma_gather indices are int16, so the 50176-row table is addressed in a
    "lo" half (rows < 32768) and a "hi" half; each (block, graph) edge group
    is split by source-row half on the host.

Self-contained: hardcodes the problem shapes; host-side numpy does edge
preprocessing, the device kernel does all per-layer compute.
"""

import math
import os

import numpy as np

F = 128
DEPTH = 4
P = 128

LAST_INFO = {}


class _Cfg:
    def __init__(self, n_nodes, n_cores=8, bpc=7, split=32768):
        assert n_nodes % n_cores == 0
        self.N = n_nodes
        self.NCORES = n_cores
        self.SH = n_nodes // n_cores              # real nodes per core
        self.NBLK = math.ceil(self.SH / P)        # 128-node blocks per core
        self.SHPAD = self.NBLK * P                # padded shard rows
        self.TBL = self.SHPAD * n_cores           # full table rows
        self.SPLIT = min(split, self.TBL)         # lo/hi boundary (int16 idx)
        self.BPC = min(bpc, self.NBLK)            # blocks per supergroup
        self.NSG = math.ceil(self.NBLK / bpc)     # supergroups
        # dma_gather HW ring limit: <=1024 idxs (8 chunks) per call
        self.CALL_CAP = int(os.environ.get("GCN_CALL_CAP", "8"))
        self.sgs = [
            list(range(s * bpc, min((s + 1) * bpc, self.NBLK)))
            for s in range(self.NSG)
        ]


def _prep_graph(edge_index, cfg):
    """Per-edge (core, blk, half, idx16, dst_local, norm) incl. self-loops."""
    src = np.asarray(edge_index[0]).astype(np.int64)
    dst = np.asarray(edge_index[1]).astype(np.int64)
    loop = np.arange(cfg.N, dtype=np.int64)
    src = np.concatenate([src, loop])
    dst = np.concatenate([dst, loop])
    deg = np.bincount(dst, minlength=cfg.N).astype(np.float64)
    dinv = np.zeros(cfg.N, np.float64)
    nz = deg > 0
    dinv[nz] = 1.0 / np.sqrt(deg[nz])
    norm = (dinv[src] * dinv[dst]).astype(np.float32)

    core = dst // cfg.SH
    loc = dst - core * cfg.SH
    blk = loc // P
    dl = (loc % P).astype(np.float32)
    scero = src // cfg.SH
    tr = scero * cfg.SHPAD + (src - scero * cfg.SH)      # table row of source
    half = (tr >= cfg.SPLIT).astype(np.int64)
    idxv = (tr - half * cfg.SPLIT).astype(np.int32)
    assert idxv.max() < 32768
    return core, blk, half, idxv, dl, norm


def _host_prep(x, edge_index, control_edge_index, cfg):
    """Build per-core device input arrays + per-group chunk counts."""
    graphs = [_prep_graph(edge_index, cfg), _prep_graph(control_edge_index, cfg)]

    # group counts per (core, blk, half) for each graph
    counts = []
    orders = []
    offsets = []
    for g in range(2):
        core, blk, half, idxv, dl, norm = graphs[g]
        key = (core * cfg.NBLK + blk) * 2 + half
        cnt = np.bincount(key, minlength=cfg.NCORES * cfg.NBLK * 2)
        cnt = cnt.reshape(cfg.NCORES, cfg.NBLK, 2)
        counts.append(cnt)
        order = np.argsort(key, kind="stable")
        orders.append(order)
        off = np.zeros(cfg.NCORES * cfg.NBLK * 2 + 1, np.int64)
        np.cumsum(cnt.ravel(), out=off[1:])
        offsets.append(off)

    # common (max-over-cores) chunk counts per (graph, blk, half)
    K = []
    for g in range(2):
        K.append(np.ceil(counts[g].max(axis=0) / P).astype(np.int64))  # [NBLK,2]

    # global chunk order: sg -> (g,h) -> blk in sg -> chunks.
    # Each (sg,g,h) "group" gets one SBUF tile but is gathered with
    # multiple dma_gather calls of <= CAP chunks (1024-idx ring limit).
    CAP = cfg.CALL_CAP
    slot_off = {}
    groups = []   # (sg, g, h, tile_start, ck, [(cstart, k), ...])
    pos = 0
    for sg_i, blocks in enumerate(cfg.sgs):
        for g in range(2):
            for h in range(2):
                tile_start = pos
                for b in blocks:
                    k = int(K[g][b, h])
                    slot_off[(g, b, h)] = pos * P
                    pos += k
                ck = pos - tile_start
                calls = []
                c = tile_start
                while c < pos:
                    k = min(CAP, pos - c)
                    calls.append((c, k))
                    c += k
                groups.append((sg_i, g, h, tile_start, ck, calls))
    nch = pos
    total_slots = nch * P

    # per-core slot arrays
    idx16s, paramss = [], []
    for r in range(cfg.NCORES):
        idx_slots = np.zeros(total_slots, np.int16)
        dl_slots = np.full(total_slots, 255.0, np.float32)
        norm_slots = np.zeros(total_slots, np.float32)
        for g in range(2):
            core, blk, half, idxv, dl, norm = graphs[g]
            order = orders[g]
            off = offsets[g]
            for b in range(cfg.NBLK):
                for h in range(2):
                    gi = (r * cfg.NBLK + b) * 2 + h
                    lo, hi_ = off[gi], off[gi + 1]
                    sel = order[lo:hi_]
                    n = hi_ - lo
                    s0 = slot_off[(g, b, h)]
                    idx_slots[s0:s0 + n] = idxv[sel].astype(np.int16)
                    dl_slots[s0:s0 + n] = dl[sel]
                    norm_slots[s0:s0 + n] = norm[sel]
        # wrap indices: per call, element i -> [i%16, i//16]
        wrapped = np.zeros((16, total_slots // 16), np.int16)
        for (_, _, _, _, _, calls) in groups:
            for (cs, k) in calls:
                s0, L = cs * P, k * P
                wrapped[:, s0 // 16:(s0 + L) // 16] = (
                    idx_slots[s0:s0 + L].reshape(L // 16, 16).T
                )
        idx16s.append(np.tile(wrapped, (8, 1)))
        params = np.empty((P, 2 * nch), np.float32)
        params[:, 0::2] = dl_slots.reshape(nch, P).T
        params[:, 1::2] = norm_slots.reshape(nch, P).T
        paramss.append(params)

    # padded fp16 node table for layer 0
    xpad = np.zeros((cfg.TBL, F), np.float16)
    xr = np.asarray(x, np.float32).reshape(cfg.NCORES, cfg.SH, F)
    for r in range(cfg.NCORES):
        xpad[r * cfg.SHPAD:r * cfg.SHPAD + cfg.SH] = xr[r].astype(np.float16)

    meta = dict(K=K, groups=groups, slot_off=slot_off, nch=nch,
                total_slots=total_slots)
    return idx16s, paramss, xpad, meta


def _build_program(cfg, meta, depth, has_bias):
    import concourse.bacc as bacc
    import concourse.mybir as mybir
    import concourse.tile as tile

    dtH = mybir.dt.float16
    dt32 = mybir.dt.float32
    AT = mybir.AluOpType
    K = meta["K"]
    groups = meta["groups"]
    slot_off = meta["slot_off"]
    nch = meta["nch"]

    nc = bacc.Bacc(
        "TRN2", debug=False, num_devices=cfg.NCORES,
        dynamic_dma_scratch_size=int(
            os.environ.get("GCN_DMA_SCRATCH", "16384")),
    )

    xpad_t = nc.dram_tensor("xpad", [cfg.TBL, F], dtH, kind="ExternalInput")
    idx_t = nc.dram_tensor("idx16", [P, meta["total_slots"] // 16],
                           mybir.dt.int16, kind="ExternalInput")
    par_t = nc.dram_tensor("params", [P, 2 * nch], dt32, kind="ExternalInput")
    w1_t = nc.dram_tensor("w1", [F, depth * F], dtH, kind="ExternalInput")
    w2_t = nc.dram_tensor("w2", [F, depth * F], dtH, kind="ExternalInput")
    iota_t = nc.dram_tensor("iota", [P, P], dtH, kind="ExternalInput")
    if has_bias:
        bsum_t = nc.dram_tensor("bsum", [depth, F], dtH, kind="ExternalInput")
    out_t = nc.dram_tensor("out", [cfg.SHPAD, F], dt32, kind="ExternalOutput")

    with tile.TileContext(nc) as tc:
        with (
            tc.tile_pool(name="const", bufs=1) as cpool,
            tc.tile_pool(name="gather", bufs=2) as gpool,
            tc.tile_pool(name="sel", bufs=8) as spool,
            tc.tile_pool(name="msb", bufs=4) as mpool,
            tc.tile_pool(name="xn", bufs=3) as xpool,
            tc.tile_pool(name="pm", bufs=4, space="PSUM") as pmpool,
            tc.tile_pool(name="po", bufs=2, space="PSUM") as popool,
            tc.tile_pool(name="shard", bufs=2, space="DRAM") as shpool,
            tc.tile_pool(name="table", bufs=2, space="DRAM") as tbpool,
        ):
            # resident constants
            idx_sb = cpool.tile([P, meta["total_slots"] // 16], mybir.dt.int16)
            nc.sync.dma_start(out=idx_sb[:], in_=idx_t[:])
            par_sb = cpool.tile([P, 2 * nch], dt32)
            nc.sync.dma_start(out=par_sb[:], in_=par_t[:])
            iota_sb = cpool.tile([P, P], dtH)
            nc.sync.dma_start(out=iota_sb[:], in_=iota_t[:])
            # weights as [F, depth*F]: layer l at free cols [l*F, (l+1)*F)
            w1_sb = cpool.tile([F, depth * F], dtH)
            w2_sb = cpool.tile([F, depth * F], dtH)
            nc.sync.dma_start(out=w1_sb[:], in_=w1_t[:])
            nc.sync.dma_start(out=w2_sb[:], in_=w2_t[:])
            if has_bias:
                bs_sb = cpool.tile([depth, F], dtH)
                nc.sync.dma_start(out=bs_sb[:], in_=bsum_t[:])
                ones_sb = cpool.tile([1, F], dtH)
                nc.vector.memset(ones_sb[:], 1.0)

            prev_table = None
            for l in range(depth):
                if l < depth - 1:
                    shard = shpool.tile([cfg.SHPAD, F], dtH, tag="shard", name="shard")

                for sg_i, blocks in enumerate(cfg.sgs):
                    # gather calls for this supergroup
                    gts = {}
                    for (sgj, g, h, ts, ck, calls) in groups:
                        if sgj != sg_i or ck == 0:
                            continue
                        gt = gpool.tile([P, ck, F], dtH, tag=f"g{g}h{h}", name=f"gt{g}{h}")
                        if l == 0:
                            src_ap = (xpad_t[0:cfg.SPLIT, :] if h == 0
                                      else xpad_t[cfg.SPLIT:cfg.TBL, :])
                        else:
                            src_ap = (prev_table[0:cfg.SPLIT, :] if h == 0
                                      else prev_table[cfg.SPLIT:cfg.TBL, :])
                        for (cs, k) in calls:
                            L = k * P
                            nc.gpsimd.dma_gather(
                                gt[:, cs - ts:cs - ts + k, :], src_ap,
                                idx_sb[:, cs * 8:cs * 8 + L // 16],
                                L, L, F,
                            )
                        gts[(g, h)] = (gt, ts)

                    for b in blocks:
                        psum_m = [
                            pmpool.tile([P, P], dt32, tag="pm", name="pm0"),
                            pmpool.tile([P, P], dt32, tag="pm", name="pm1"),
                        ]
                        for g in range(2):
                            tot = int(K[g][b, 0] + K[g][b, 1])
                            done = 0
                            for h in range(2):
                                kbh = int(K[g][b, h])
                                if kbh == 0:
                                    continue
                                gt, cs = gts[(g, h)]
                                c0 = slot_off[(g, b, h)] // P - cs
                                for c in range(kbh):
                                    pc = slot_off[(g, b, h)] // P + c
                                    sel = spool.tile([P, P], dtH, tag="sel", name="sel")
                                    nc.vector.tensor_scalar(
                                        out=sel[:], in0=iota_sb[:],
                                        scalar1=par_sb[:, 2 * pc:2 * pc + 1],
                                        scalar2=par_sb[:, 2 * pc + 1:2 * pc + 2],
                                        op0=AT.is_equal, op1=AT.mult,
                                    )
                                    nc.tensor.matmul(
                                        out=psum_m[g][:],
                                        lhsT=gt[:, c0 + c, :],
                                        rhs=sel[:],
                                        start=(done == 0),
                                        stop=(done == tot - 1),
                                    )
                                    done += 1
                        m_sb = [mpool.tile([P, P], dtH, tag="m", name="m0"),
                                mpool.tile([P, P], dtH, tag="m", name="m1")]
                        nc.vector.tensor_copy(out=m_sb[0][:], in_=psum_m[0][:])
                        nc.vector.tensor_copy(out=m_sb[1][:], in_=psum_m[1][:])
                        pout = popool.tile([P, P], dt32, tag="po", name="pout")
                        nc.tensor.matmul(out=pout[:], lhsT=m_sb[0][:],
                                         rhs=w1_sb[:, l * F:(l + 1) * F],
                                         start=True, stop=False)
                        nc.tensor.matmul(out=pout[:], lhsT=m_sb[1][:],
                                         rhs=w2_sb[:, l * F:(l + 1) * F],
                                         start=False, stop=not has_bias)
                        if has_bias:
                            nc.tensor.matmul(out=pout[:], lhsT=ones_sb[:1, :],
                                             rhs=bs_sb[l:l + 1, :],
                                             start=False, stop=True)
                        if l < depth - 1:
                            xn = xpool.tile([P, P], dtH, tag="xn", name="xn")
                            nc.scalar.activation(
                                out=xn[:], in_=pout[:],
                                func=mybir.ActivationFunctionType.Relu)
                            nc.sync.dma_start(
                                out=shard[b * P:(b + 1) * P, :][:], in_=xn[:])
                        else:
                            xn = xpool.tile([P, P], dt32, tag="xnf", name="xnf")
                            nc.vector.tensor_copy(out=xn[:], in_=pout[:])
                            nc.sync.dma_start(
                                out=out_t[b * P:(b + 1) * P, :], in_=xn[:])

                if l < depth - 1:
                    table = tbpool.tile([cfg.TBL, F], dtH, tag="table", name="table", addr_space="Shared")
                    nc.gpsimd.collective_compute(
                        "AllGather",
                        mybir.AluOpType.bypass,
                        replica_groups=[list(range(cfg.NCORES))],
                        ins=[shard.opt()],
                        outs=[table.opt()],
                    )
                    prev_table = table

    nc.compile()
    return nc


def _run(x, edge_index, control_edge_index, conv_w, conv_b, ctrl_w, ctrl_b,
         cfg, trace=False):
    from concourse.bass_utils import run_bass_kernel_spmd

    depth = int(np.asarray(conv_w).shape[0])
    idx16s, paramss, xpad, meta = _host_prep(x, edge_index,
                                             control_edge_index, cfg)
    bsum = (np.asarray(conv_b, np.float32)
            + np.asarray(ctrl_b, np.float32))
    has_bias = bool(np.any(bsum))
    nc = _build_program(cfg, meta, depth, has_bias)

    w1 = (np.asarray(conv_w, np.float32).transpose(1, 0, 2)
          .reshape(F, depth * F).astype(np.float16))
    w2 = (np.asarray(ctrl_w, np.float32).transpose(1, 0, 2)
          .reshape(F, depth * F).astype(np.float16))
    iota = np.tile(np.arange(P, dtype=np.float16), (P, 1))

    in_maps = []
    for r in range(cfg.NCORES):
        m = {"xpad": xpad, "idx16": idx16s[r], "params": paramss[r],
             "w1": w1, "w2": w2, "iota": iota}
        if has_bias:
            m["bsum"] = bsum.astype(np.float16)
        in_maps.append(m)

    try:
        res = run_bass_kernel_spmd(nc, in_maps, list(range(cfg.NCORES)),
                                   trace=trace)
    except Exception:
        if not trace:
            raise
        res = run_bass_kernel_spmd(nc, in_maps, list(range(cfg.NCORES)),
                                   trace=False)
    LAST_INFO.clear()
    LAST_INFO["exec_time_ns"] = res.exec_time_ns
    LAST_INFO["mean_exec_time_ns"] = res.mean_exec_time_ns
    LAST_INFO["profile_json"] = res.profile_json

    out = np.empty((cfg.N, F), np.float32)
    for r in range(cfg.NCORES):
        out[r * cfg.SH:(r + 1) * cfg.SH] = res.results[r]["out"][:cfg.SH]
    return out


def kernel(x, edge_index, control_edge_index, conv_w, conv_b, ctrl_w, ctrl_b):
    cfg = _Cfg(int(np.asarray(x).shape[0]))
    trace = os.environ.get("GCN_TRACE", "0") == "1"
    return _run(x, edge_index, control_edge_index, conv_w, conv_b,
                ctrl_w, ctrl_b, cfg, trace=trace)


# revision 20
# speedup vs baseline: 1.8774x; 1.8774x over previous
"""GCNBlock (2-graph GCN, depth 4) on 8 Trainium2 NeuronCores.

Strategy (graph/data parallel, per sharding hint):
  - Nodes row-sharded 8 ways (6250/core, padded to 6272 = 49 blocks of 128).
  - Per layer: every core aggregates messages for its own 6250 dst nodes by
    DMA-gathering per-edge source rows (fp16) from a replicated node-feature
    table in HBM, reduces them into per-block sums with PE matmuls against
    per-chunk selection matrices (built on DVE: (iota == dst_local) * norm),
    applies the two [F,F] GEMMs + bias + ReLU, then AllGathers the new
    feature shard so every core has the full table for the next layer.
  - dma_gather indices are int16, so the 50176-row table is addressed in a
    "lo" half (rows < 32768) and a "hi" half; each (block, graph) edge group
    is split by source-row half on the host.

Self-contained: hardcodes the problem shapes; host-side numpy does edge
preprocessing, the device kernel does all per-layer compute.
"""

import math
import os

import numpy as np

F = 128
DEPTH = 4
P = 128

LAST_INFO = {}


class _Cfg:
    def __init__(self, n_nodes, n_cores=8, bpc=7, split=32768):
        assert n_nodes % n_cores == 0
        self.N = n_nodes
        self.NCORES = n_cores
        self.SH = n_nodes // n_cores              # real nodes per core
        self.NBLK = math.ceil(self.SH / P)        # 128-node blocks per core
        self.SHPAD = self.NBLK * P                # padded shard rows
        self.TBL = self.SHPAD * n_cores           # full table rows
        self.SPLIT = min(split, self.TBL)         # lo/hi boundary (int16 idx)
        self.BPC = min(bpc, self.NBLK)            # blocks per supergroup
        self.NSG = math.ceil(self.NBLK / bpc)     # supergroups
        # dma_gather HW ring limit: <=1024 idxs (8 chunks) per call
        self.CALL_CAP = int(os.environ.get("GCN_CALL_CAP", "8"))
        self.sgs = [
            list(range(s * bpc, min((s + 1) * bpc, self.NBLK)))
            for s in range(self.NSG)
        ]


def _prep_graph(edge_index, cfg):
    """Per-edge (core, blk, half, idx16, dst_local, norm) incl. self-loops."""
    src = np.asarray(edge_index[0]).astype(np.int64)
    dst = np.asarray(edge_index[1]).astype(np.int64)
    loop = np.arange(cfg.N, dtype=np.int64)
    src = np.concatenate([src, loop])
    dst = np.concatenate([dst, loop])
    deg = np.bincount(dst, minlength=cfg.N).astype(np.float64)
    dinv = np.zeros(cfg.N, np.float64)
    nz = deg > 0
    dinv[nz] = 1.0 / np.sqrt(deg[nz])
    norm = (dinv[src] * dinv[dst]).astype(np.float32)

    core = dst // cfg.SH
    loc = dst - core * cfg.SH
    blk = loc // P
    dl = (loc % P).astype(np.float32)
    scero = src // cfg.SH
    tr = scero * cfg.SHPAD + (src - scero * cfg.SH)      # table row of source
    half = (tr >= cfg.SPLIT).astype(np.int64)
    idxv = (tr - half * cfg.SPLIT).astype(np.int32)
    assert idxv.max() < 32768
    return core, blk, half, idxv, dl, norm


def _host_prep(x, edge_index, control_edge_index, cfg):
    """Build per-core device input arrays + per-group chunk counts."""
    graphs = [_prep_graph(edge_index, cfg), _prep_graph(control_edge_index, cfg)]

    # group counts per (core, blk, half) for each graph
    counts = []
    orders = []
    offsets = []
    for g in range(2):
        core, blk, half, idxv, dl, norm = graphs[g]
        key = (core * cfg.NBLK + blk) * 2 + half
        cnt = np.bincount(key, minlength=cfg.NCORES * cfg.NBLK * 2)
        cnt = cnt.reshape(cfg.NCORES, cfg.NBLK, 2)
        counts.append(cnt)
        order = np.argsort(key, kind="stable")
        orders.append(order)
        off = np.zeros(cfg.NCORES * cfg.NBLK * 2 + 1, np.int64)
        np.cumsum(cnt.ravel(), out=off[1:])
        offsets.append(off)

    # common (max-over-cores) chunk counts per (graph, blk, half)
    K = []
    for g in range(2):
        K.append(np.ceil(counts[g].max(axis=0) / P).astype(np.int64))  # [NBLK,2]

    # global chunk order: blk -> (g,h) -> chunks.  Each (blk,g,h) "group"
    # gets one SBUF tile, gathered with dma_gather calls of <= CAP chunks
    # (HW ring limit: 1024 idxs per call).
    CAP = cfg.CALL_CAP
    slot_off = {}
    groups = []   # (b, g, h, tile_start, ck, [(cstart, k), ...])
    pos = 0
    for b in range(cfg.NBLK):
        for g in range(2):
            for h in range(2):
                tile_start = pos
                ck = int(K[g][b, h])
                slot_off[(g, b, h)] = pos * P
                pos += ck
                calls = []
                c = tile_start
                while c < pos:
                    k = min(CAP, pos - c)
                    calls.append((c, k))
                    c += k
                groups.append((b, g, h, tile_start, ck, calls))
    nch = pos
    total_slots = nch * P

    # per-core slot arrays
    idx16s, paramss = [], []
    for r in range(cfg.NCORES):
        idx_slots = np.zeros(total_slots, np.int16)
        dl_slots = np.full(total_slots, 255.0, np.float32)
        norm_slots = np.zeros(total_slots, np.float32)
        for g in range(2):
            core, blk, half, idxv, dl, norm = graphs[g]
            order = orders[g]
            off = offsets[g]
            for b in range(cfg.NBLK):
                for h in range(2):
                    gi = (r * cfg.NBLK + b) * 2 + h
                    lo, hi_ = off[gi], off[gi + 1]
                    sel = order[lo:hi_]
                    n = hi_ - lo
                    s0 = slot_off[(g, b, h)]
                    idx_slots[s0:s0 + n] = idxv[sel].astype(np.int16)
                    dl_slots[s0:s0 + n] = dl[sel]
                    norm_slots[s0:s0 + n] = norm[sel]
        # wrap indices: per call, element i -> [i%16, i//16]
        wrapped = np.zeros((16, total_slots // 16), np.int16)
        for (_, _, _, _, _, calls) in groups:
            for (cs, k) in calls:
                s0, L = cs * P, k * P
                wrapped[:, s0 // 16:(s0 + L) // 16] = (
                    idx_slots[s0:s0 + L].reshape(L // 16, 16).T
                )
        idx16s.append(np.tile(wrapped, (8, 1)))
        dst16 = dl_slots.reshape(nch, P).T.astype(np.float16)
        nrm16 = norm_slots.reshape(nch, P).T.astype(np.float16)
        paramss.append((dst16, nrm16))

    # padded fp16 node table for layer 0
    xpad = np.zeros((cfg.TBL, F), np.float16)
    xr = np.asarray(x, np.float32).reshape(cfg.NCORES, cfg.SH, F)
    for r in range(cfg.NCORES):
        xpad[r * cfg.SHPAD:r * cfg.SHPAD + cfg.SH] = xr[r].astype(np.float16)

    meta = dict(K=K, groups=groups, slot_off=slot_off, nch=nch,
                total_slots=total_slots)
    return idx16s, paramss, xpad, meta


def _build_program(cfg, meta, depth, has_bias):
    import concourse.bacc as bacc
    import concourse.mybir as mybir
    import concourse.tile as tile

    dtH = mybir.dt.float16
    dt32 = mybir.dt.float32
    AT = mybir.AluOpType
    K = meta["K"]
    groups = meta["groups"]
    slot_off = meta["slot_off"]
    nch = meta["nch"]

    NQ = int(os.environ.get("GCN_NQUEUES", "4"))
    nc = bacc.Bacc(
        "TRN2", debug=False, num_devices=cfg.NCORES,
        num_swdge_queues=NQ,
        dynamic_dma_scratch_size=int(
            os.environ.get("GCN_DMA_SCRATCH", "16384")),
    )

    xpad_t = nc.dram_tensor("xpad", [cfg.TBL, F], dtH, kind="ExternalInput")
    idx_t = nc.dram_tensor("idx16", [P, meta["total_slots"] // 16],
                           mybir.dt.int16, kind="ExternalInput")
    dst_t = nc.dram_tensor("dstloc", [P, nch], dtH, kind="ExternalInput")
    nrm_t = nc.dram_tensor("normv", [P, nch], dtH, kind="ExternalInput")
    CAP = cfg.CALL_CAP
    iotar_t = nc.dram_tensor("iotar", [P, CAP * P], dtH, kind="ExternalInput")
    w1_t = nc.dram_tensor("w1", [F, depth * F], dtH, kind="ExternalInput")
    w2_t = nc.dram_tensor("w2", [F, depth * F], dtH, kind="ExternalInput")
    if has_bias:
        bsum_t = nc.dram_tensor("bsum", [depth, F], dtH, kind="ExternalInput")
    out_t = nc.dram_tensor("out", [cfg.SHPAD, F], dt32, kind="ExternalOutput")

    with tile.TileContext(nc) as tc:
        with (
            tc.tile_pool(name="const", bufs=1) as cpool,
            tc.tile_pool(name="gather", bufs=3) as gpool,
            tc.tile_pool(name="sel", bufs=8) as spool,
            tc.tile_pool(name="msb", bufs=4) as mpool,
            tc.tile_pool(name="xn", bufs=3) as xpool,
            tc.tile_pool(name="pm", bufs=4, space="PSUM") as pmpool,
            tc.tile_pool(name="po", bufs=2, space="PSUM") as popool,
            tc.tile_pool(name="shard", bufs=2, space="DRAM") as shpool,
            tc.tile_pool(name="table", bufs=2, space="DRAM") as tbpool,
        ):
            # resident constants
            idx_sb = cpool.tile([P, meta["total_slots"] // 16], mybir.dt.int16)
            nc.sync.dma_start(out=idx_sb[:], in_=idx_t[:])
            dst_sb = cpool.tile([P, nch], dtH)
            nc.sync.dma_start(out=dst_sb[:], in_=dst_t[:])
            nrm_sb = cpool.tile([P, nch], dtH)
            nc.sync.dma_start(out=nrm_sb[:], in_=nrm_t[:])
            iotar_sb = cpool.tile([P, CAP, P], dtH)
            nc.sync.dma_start(out=iotar_sb[:], in_=iotar_t[:])
            # weights as [F, depth*F]: layer l at free cols [l*F, (l+1)*F)
            w1_sb = cpool.tile([F, depth * F], dtH)
            w2_sb = cpool.tile([F, depth * F], dtH)
            nc.sync.dma_start(out=w1_sb[:], in_=w1_t[:])
            nc.sync.dma_start(out=w2_sb[:], in_=w2_t[:])
            if has_bias:
                bs_sb = cpool.tile([depth, F], dtH)
                nc.sync.dma_start(out=bs_sb[:], in_=bsum_t[:])
                ones_sb = cpool.tile([1, F], dtH)
                nc.vector.memset(ones_sb[:], 1.0)

            qrr = [0]
            prev_table = None
            for l in range(depth):
                if l < depth - 1:
                    shard = shpool.tile([cfg.SHPAD, F], dtH, tag="shard", name="shard")

                gmap = {(b_, g_, h_): (ts_, ck_, calls_)
                        for (b_, g_, h_, ts_, ck_, calls_) in groups}
                for b in range(cfg.NBLK):
                    psum_m = [
                        pmpool.tile([P, P], dt32, tag="pm", name="pm0"),
                        pmpool.tile([P, P], dt32, tag="pm", name="pm1"),
                    ]
                    for g in range(2):
                        tot = int(K[g][b, 0] + K[g][b, 1])
                        done = 0
                        for h in range(2):
                            ts, ck, calls = gmap[(b, g, h)]
                            if ck == 0:
                                continue
                            gt = gpool.tile([P, ck, F], dtH, tag=f"g{g}h{h}",
                                            name=f"gt{g}{h}")
                            if l == 0:
                                src_ap = (xpad_t[0:cfg.SPLIT, :] if h == 0
                                          else xpad_t[cfg.SPLIT:cfg.TBL, :])
                            else:
                                src_ap = (prev_table[0:cfg.SPLIT, :] if h == 0
                                          else prev_table[cfg.SPLIT:cfg.TBL, :])
                            for (cs, k) in calls:
                                L = k * P
                                nc.gpsimd.dma_gather(
                                    gt[:, cs - ts:cs - ts + k, :], src_ap,
                                    idx_sb[:, cs * 8:cs * 8 + L // 16],
                                    L, L, F,
                                    queue_num=qrr[0] % NQ,
                                )
                                qrr[0] += 1
                                selb = spool.tile([P, CAP, P], dtH, tag="sel",
                                                  name="selb")
                                nc.vector.tensor_tensor(
                                    out=selb[:, :k, :],
                                    in0=iotar_sb[:, :k, :],
                                    in1=dst_sb[:, cs:cs + k].to_broadcast(
                                        [P, k, P]),
                                    op=AT.is_equal,
                                )
                                nc.vector.tensor_tensor(
                                    out=selb[:, :k, :],
                                    in0=selb[:, :k, :],
                                    in1=nrm_sb[:, cs:cs + k].to_broadcast(
                                        [P, k, P]),
                                    op=AT.mult,
                                )
                                for j in range(k):
                                    nc.tensor.matmul(
                                        out=psum_m[g][:],
                                        lhsT=gt[:, cs - ts + j, :],
                                        rhs=selb[:, j, :],
                                        start=(done == 0),
                                        stop=(done == tot - 1),
                                    )
                                    done += 1
                    m_sb = [mpool.tile([P, P], dtH, tag="m", name="m0"),
                            mpool.tile([P, P], dtH, tag="m", name="m1")]
                    nc.vector.tensor_copy(out=m_sb[0][:], in_=psum_m[0][:])
                    nc.vector.tensor_copy(out=m_sb[1][:], in_=psum_m[1][:])
                    pout = popool.tile([P, P], dt32, tag="po", name="pout")
                    nc.tensor.matmul(out=pout[:], lhsT=m_sb[0][:],
                                     rhs=w1_sb[:, l * F:(l + 1) * F],
                                     start=True, stop=False)
                    nc.tensor.matmul(out=pout[:], lhsT=m_sb[1][:],
                                     rhs=w2_sb[:, l * F:(l + 1) * F],
                                     start=False, stop=not has_bias)
                    if has_bias:
                        nc.tensor.matmul(out=pout[:], lhsT=ones_sb[:1, :],
                                         rhs=bs_sb[l:l + 1, :],
                                         start=False, stop=True)
                    if l < depth - 1:
                        xn = xpool.tile([P, P], dtH, tag="xn", name="xn")
                        nc.scalar.activation(
                            out=xn[:], in_=pout[:],
                            func=mybir.ActivationFunctionType.Relu)
                        nc.sync.dma_start(
                            out=shard[b * P:(b + 1) * P, :][:], in_=xn[:])
                    else:
                        xn = xpool.tile([P, P], dt32, tag="xnf", name="xnf")
                        nc.vector.tensor_copy(out=xn[:], in_=pout[:])
                        nc.sync.dma_start(
                            out=out_t[b * P:(b + 1) * P, :], in_=xn[:])

                if l < depth - 1:
                    table = tbpool.tile([cfg.TBL, F], dtH, tag="table", name="table", addr_space="Shared")
                    nc.gpsimd.collective_compute(
                        "AllGather",
                        mybir.AluOpType.bypass,
                        replica_groups=[list(range(cfg.NCORES))],
                        ins=[shard.opt()],
                        outs=[table.opt()],
                    )
                    prev_table = table

    nc.compile()
    return nc


def _run(x, edge_index, control_edge_index, conv_w, conv_b, ctrl_w, ctrl_b,
         cfg, trace=False):
    from concourse.bass_utils import run_bass_kernel_spmd

    depth = int(np.asarray(conv_w).shape[0])
    idx16s, paramss, xpad, meta = _host_prep(x, edge_index,
                                             control_edge_index, cfg)
    bsum = (np.asarray(conv_b, np.float32)
            + np.asarray(ctrl_b, np.float32))
    has_bias = bool(np.any(bsum))
    nc = _build_program(cfg, meta, depth, has_bias)

    w1 = (np.asarray(conv_w, np.float32).transpose(1, 0, 2)
          .reshape(F, depth * F).astype(np.float16))
    w2 = (np.asarray(ctrl_w, np.float32).transpose(1, 0, 2)
          .reshape(F, depth * F).astype(np.float16))
    cap = cfg.CALL_CAP
    iotar = np.tile(np.arange(P, dtype=np.float16), (P, cap))

    in_maps = []
    for r in range(cfg.NCORES):
        dst16, nrm16 = paramss[r]
        m = {"xpad": xpad, "idx16": idx16s[r], "dstloc": dst16,
             "normv": nrm16, "w1": w1, "w2": w2, "iotar": iotar}
        if has_bias:
            m["bsum"] = bsum.astype(np.float16)
        in_maps.append(m)

    try:
        res = run_bass_kernel_spmd(nc, in_maps, list(range(cfg.NCORES)),
                                   trace=trace)
    except Exception:
        if not trace:
            raise
        res = run_bass_kernel_spmd(nc, in_maps, list(range(cfg.NCORES)),
                                   trace=False)
    LAST_INFO.clear()
    LAST_INFO["exec_time_ns"] = res.exec_time_ns
    LAST_INFO["mean_exec_time_ns"] = res.mean_exec_time_ns
    LAST_INFO["profile_json"] = res.profile_json

    out = np.empty((cfg.N, F), np.float32)
    for r in range(cfg.NCORES):
        out[r * cfg.SH:(r + 1) * cfg.SH] = res.results[r]["out"][:cfg.SH]
    return out


def kernel(x, edge_index, control_edge_index, conv_w, conv_b, ctrl_w, ctrl_b):
    cfg = _Cfg(int(np.asarray(x).shape[0]))
    trace = os.environ.get("GCN_TRACE", "0") == "1"
    return _run(x, edge_index, control_edge_index, conv_w, conv_b,
                ctrl_w, ctrl_b, cfg, trace=trace)
